# revision 1
# baseline (speedup 1.0000x reference)
"""Trainium2 Bass kernel for nn_Attention_35588099015470 (v3, fp16).

Full transformer attention block: LoRA linears (folded host-side) + RoPE +
causal SDPA + output projection, B=2 T=2048 C=2048 H=16 D=128.

Design:
- fp16 operands everywhere, fp32 PSUM accumulation (end-to-end rel err
  ~5e-4 vs gate 2e-2).
- q/k/v SBUF-resident between projection and attention, per-batch tiles so
  batch 1's projections don't false-serialize against batch 0's attention.
- Attention per (batch, head) pair runs query tiles in halves (2+2) with
  jc-outer loops; the two query tiles of a half share one PSUM score group
  per key chunk and one grouped exp on the scalar engine. Score matmuls and
  exp skip the fully-masked leading columns of diagonal chunks; the causal
  mask is a DVE multiply on the exp'd probabilities.
- Softmax denominators: DVE fp16 accumulation over key chunks + one
  [128,1]-stationary ones-matmul per query tile; eager per-qt fast
  reciprocal + gpsimd broadcast + normalize so PV PSUM banks free early and
  each pair's AllToAll fires immediately.
- Output projection in two 8-column waves; wave 1 pre-accumulates the
  head-0..7 kc blocks (whose AllToAlls land early) while the last AllToAll
  is in flight. Head assignment is hl-major (core c owns heads c, c+8) so
  kc order equals head index.

Biases are guaranteed zero by the problem's setup_inputs and the mask is the
causal tril; if either assumption is violated at runtime we fall back to a
host reference implementation so the kernel stays correct on any input.
"""
import sys

sys.path.insert(0, "/opt/trn_rl_repo")

import numpy as np
from contextlib import ExitStack

import concourse.tile as tile
from concourse import bacc, mybir
from concourse.bass_utils import run_bass_kernel_spmd

dt = mybir.dt
F16 = dt.float16
F32 = dt.float32

B, T, C, H, R = 2, 2048, 2048, 16, 8
D = C // H            # 128
NCORES = 8
HPC = H // NCORES     # heads per core = 2
P = 128
BT = B * T            # 4096
KC = C // P           # 16 contraction chunks
QT = T // 512         # 4 query tiles per (b, hl)
SCALE = 1.0 / float(np.sqrt(D))

_PROGRAM = None


def _build_program():
    nc = bacc.Bacc("TRN2", target_bir_lowering=False, debug=False,
                   num_devices=NCORES)

    xT_d = nc.dram_tensor("xT", [C, BT], F16, kind="ExternalInput")
    wqT_d = nc.dram_tensor("wqT", [C, HPC * D], F16, kind="ExternalInput")
    wkT_d = nc.dram_tensor("wkT", [C, HPC * D], F16, kind="ExternalInput")
    wvT_d = nc.dram_tensor("wvT", [C, HPC * D], F16, kind="ExternalInput")
    pwB_d = nc.dram_tensor("pwB", [KC, P, KC, P], F16, kind="ExternalInput")
    cosA_d = nc.dram_tensor("cosA", [P, BT], F32, kind="ExternalInput")
    sinA_d = nc.dram_tensor("sinA", [P, BT], F32, kind="ExternalInput")
    dmask_d = nc.dram_tensor("dmask", [4, P, 512], F16, kind="ExternalInput")
    ident_d = nc.dram_tensor("ident", [P, P], F16, kind="ExternalInput")

    outT_d = nc.dram_tensor("outT", [C, 512], F16, kind="ExternalOutput")

    with tile.TileContext(nc) as tc, ExitStack() as ctx:
        dram = ctx.enter_context(tc.tile_pool(name="dram", bufs=1, space="DRAM"))
        chs = [[dram.tile([NCORES, D, 256], F16, name=f"ch_{b}_{hl}")
                for hl in range(HPC)] for b in range(B)]
        yosA = dram.tile([B, HPC, NCORES * D, 256], F16, name="yosA")
        yos = [[yosA[b, hl] for hl in range(HPC)] for b in range(B)]

        # ---- persistent SBUF pools ----
        cst = ctx.enter_context(tc.tile_pool(name="cst", bufs=1))
        res = ctx.enter_context(tc.tile_pool(name="res", bufs=1))
        xp = ctx.enter_context(tc.tile_pool(name="xp", bufs=2))
        csp = ctx.enter_context(tc.tile_pool(name="csp", bufs=2))
        ppool = ctx.enter_context(tc.tile_pool(name="ppool", bufs=4))
        ycp = ctx.enter_context(tc.tile_pool(name="ycp", bufs=1))

        xT_view = xT_d.ap().rearrange("(a p) t -> p a t", p=P)

        # q weights + first x tile + first cos/sin first: the first matmul
        # depends on exactly these DMAs
        wsb = {}

        def load_w(nm, wd):
            w_sb = cst.tile([P, KC, HPC * D], F16, name=f"w{nm}_sb")
            wv_view = wd.ap().rearrange("(a p) m -> p a m", p=P)
            for g in range(4):
                nc.sync.dma_start(w_sb[:, g * 4:(g + 1) * 4, :],
                                  wv_view[:, g * 4:(g + 1) * 4, :])
            wsb[nm] = w_sb

        def load_xt(tt):
            tsl = slice(tt * 512, (tt + 1) * 512)
            xt = xp.tile([P, KC, 512], F16, name=f"xt_{tt}", tag="xt")
            for g in range(4):
                nc.sync.dma_start(xt[:, g * 4:(g + 1) * 4, :],
                                  xT_view[:, g * 4:(g + 1) * 4, tsl])
            cs_c = csp.tile([P, 512], F32, tag="csc", name=f"csc_{tt}")
            nc.sync.dma_start(cs_c[:], cosA_d.ap()[:, tsl])
            cs_s = csp.tile([P, 512], F32, tag="css", name=f"css_{tt}")
            nc.sync.dma_start(cs_s[:], sinA_d.ap()[:, tsl])
            return xt, cs_c, cs_s

        # interleave wq/x chunk loads so the first matmul (needs chunk g=0
        # of both) can start as early as possible
        wq_sb = cst.tile([P, KC, HPC * D], F16, name="wq_sb")
        wq_view = wqT_d.ap().rearrange("(a p) m -> p a m", p=P)
        xt0 = xp.tile([P, KC, 512], F16, name="xt_0", tag="xt")
        # kc=0 slices first: they alone gate the very first matmul
        nc.sync.dma_start(wq_sb[:, 0:1, :], wq_view[:, 0:1, :])
        nc.sync.dma_start(xt0[:, 0:1, :], xT_view[:, 0:1, 0:512])
        nc.sync.dma_start(wq_sb[:, 1:4, :], wq_view[:, 1:4, :])
        nc.sync.dma_start(xt0[:, 1:4, :], xT_view[:, 1:4, 0:512])
        for g in range(1, 4):
            nc.sync.dma_start(wq_sb[:, g * 4:(g + 1) * 4, :],
                              wq_view[:, g * 4:(g + 1) * 4, :])
            nc.sync.dma_start(xt0[:, g * 4:(g + 1) * 4, :],
                              xT_view[:, g * 4:(g + 1) * 4, 0:512])
        wsb["q"] = wq_sb
        cs_c0 = csp.tile([P, 512], F32, tag="csc", name="csc_0")
        nc.sync.dma_start(cs_c0[:], cosA_d.ap()[:, 0:512])
        cs_s0 = csp.tile([P, 512], F32, tag="css", name="css_0")
        nc.sync.dma_start(cs_s0[:], sinA_d.ap()[:, 0:512])
        pre_a = {0: (xt0, cs_c0, cs_s0)}
        load_w("k", wkT_d)
        load_w("v", wvT_d)

        ones_f = cst.tile([P, 1], F32, name="ones_f")
        nc.any.memset(ones_f[:], 1.0)
        ones = cst.tile([P, 1], F16, name="ones")
        nc.vector.tensor_copy(ones[:], ones_f[:])
        # touch partition_broadcast once now: the gpsimd custom-op LOAD_LIB
        # takes ~7.5us and would otherwise stall the first normalize chain
        bc_warm = cst.tile([P, 1], F32, name="bc_warm")
        nc.gpsimd.partition_broadcast(bc_warm[:], ones_f[0:1, :])
        ident = cst.tile([P, P], F16, name="ident")
        nc.sync.dma_start(ident[:], ident_d.ap())
        dmask = cst.tile([P, 4, 512], F16, name="dmask")
        for o in range(4):
            nc.sync.dma_start(dmask[:, o, :], dmask_d.ap()[o])

        # per-batch SBUF-resident projections
        qS = [res.tile([P, HPC, T], F16, name=f"qS{b}") for b in range(B)]
        kS = [res.tile([P, HPC, T], F16, name=f"kS{b}") for b in range(B)]
        vS = [res.tile([P, KC, HPC * D], F16, name=f"vS{b}") for b in range(B)]

        # zero the pT buffers once: diagonal-trimmed exp leaves their leading
        # columns untouched and the mask multiply must see finite values
        for i in range(4):
            t = ppool.tile([P, 2, 512], F16, tag="pT", name=f"pTz_{i}")
            nc.vector.memset(t[:], 0.0)

        yAB = ycp.tile([P, KC, 512], F16, name="yAB")

        def phase_a(b):
            """q/k/v projections + RoPE for batch b (4 token tiles of 512)."""
            with tc.tile_pool(name=f"pa_ps_{b}", bufs=1, space="PSUM") as pp, \
                 tc.tile_pool(name=f"pa_t_{b}", bufs=2) as tp, \
                 tc.tile_pool(name=f"pa_v_{b}", bufs=2) as vtp:
                for j in range(4):
                    tt = b * 4 + j
                    lsl = slice(j * 512, (j + 1) * 512)   # local within batch
                    if tt in pre_a:
                        xt, cs_c, cs_s = pre_a.pop(tt)
                    else:
                        xt, cs_c, cs_s = load_xt(tt)

                    for w_sb, dst in ((wsb["q"], qS[b]), (wsb["k"], kS[b])):
                        for mt in range(HPC):
                            ps = pp.tile([P, 512], F32, tag="qk", bufs=4,
                                         name=f"psA_{tt}_{mt}")
                            for kc in range(KC):
                                nc.tensor.matmul(
                                    ps[:], w_sb[:, kc, mt * P:(mt + 1) * P],
                                    xt[:, kc, :],
                                    start=(kc == 0), stop=(kc == KC - 1))
                            # rope: y = raw*cosA + halfswap(raw)*sinA
                            t1 = tp.tile([P, 512], F32, tag="t1",
                                         name=f"t1_{tt}_{mt}")
                            nc.vector.tensor_mul(t1[:], ps[:], cs_c[:])
                            t2 = tp.tile([P, 512], F32, tag="t2",
                                         name=f"t2_{tt}_{mt}")
                            nc.vector.tensor_mul(t2[0:64, :], ps[64:128, :],
                                                 cs_s[0:64, :])
                            nc.vector.tensor_mul(t2[64:128, :], ps[0:64, :],
                                                 cs_s[64:128, :])
                            nc.vector.tensor_add(dst[:, mt, lsl], t1[:], t2[:])

                    # v: transposed matmul then PE-transpose to natural layout
                    for mt in range(HPC):
                        ps = pp.tile([P, 512], F32, tag="qk", bufs=4,
                                     name=f"psVT_{tt}_{mt}")
                        for kc in range(KC):
                            nc.tensor.matmul(
                                ps[:], wsb["v"][:, kc, mt * P:(mt + 1) * P],
                                xt[:, kc, :],
                                start=(kc == 0), stop=(kc == KC - 1))
                        vT_sb = vtp.tile([P, 512], F16, tag="vts",
                                         name=f"vts_{tt}_{mt}")
                        nc.scalar.copy(vT_sb[:], ps[:])
                        for js in range(4):
                            pst = pp.tile([P, P], F16, tag="tp", bufs=2,
                                          name=f"pst_{tt}_{mt}_{js}")
                            nc.tensor.transpose(
                                pst[:], vT_sb[:, js * P:(js + 1) * P], ident[:])
                            nc.scalar.copy(
                                vS[b][:, j * 4 + js, mt * P:(mt + 1) * P],
                                pst[:])

        def phase_b(b):
            """Causal attention for batch b, pairs hl=0,1."""
            last_pt = [None]
            with tc.tile_pool(name=f"pb_ps_{b}", bufs=1, space="PSUM") as pb, \
                 tc.tile_pool(name=f"pb_n_{b}", bufs=2) as np_:
                for hl in range(HPC):
                    for half in range(2):
                        qts = (0, 1) if half == 0 else (2, 3)
                        njc = 4 * qts[-1] + 4
                        pend = []
                        acc = np_.tile([P, 2, 512], F16, tag="acc", bufs=2,
                                       name=f"acc_{b}_{hl}_{half}")
                        pvs = [pb.tile([P, 512], F32, tag="pv", bufs=3,
                                       name=f"pv_{b}_{hl}_{half}_{i}")
                               for i in range(2)]

                        def emit_scores(jc, _b=b, _hl=hl, _qts=qts,
                                        _pend=pend):
                            grp = [qt for qt in _qts if jc <= 4 * qt + 3]
                            o = jc - 4 * grp[0]
                            trim = o * P if 0 <= o <= 3 else 0
                            ps4 = pb.tile([P, 2, 512], F32, tag="sc",
                                          bufs=2, name=f"sc_{_b}_{_hl}_{jc}")
                            for i, qt in enumerate(grp):
                                tr = trim if i == 0 else 0
                                nc.tensor.matmul(
                                    ps4[:, i, tr:],
                                    kS[_b][:, _hl, jc * P:(jc + 1) * P],
                                    qS[_b][:, _hl, qt * 512 + tr:
                                           (qt + 1) * 512],
                                    start=True, stop=True)
                            _pend.append((jc, grp, trim, ps4))

                        def drain_one(_b=b, _hl=hl, _qts=qts, _half=half,
                                      _pend=pend, _acc=acc):
                            jc, grp, trim, ps4 = _pend.pop(0)
                            nq = len(grp)
                            o = jc - 4 * grp[0]
                            pT4 = ppool.tile([P, 2, 512], F16, tag="pT",
                                             name=f"pT_{_b}_{_hl}_{jc}")
                            last_pt[0] = pT4
                            pw_flat = pT4[:].rearrange("p a m -> p (a m)")
                            ps_flat = ps4[:].rearrange("p a m -> p (a m)")
                            nc.scalar.activation(
                                pw_flat[:, trim:nq * 512],
                                ps_flat[:, trim:nq * 512],
                                mybir.ActivationFunctionType.Exp, scale=SCALE)
                            if 0 <= o <= 3:
                                nc.vector.tensor_mul(pT4[:, 0, :],
                                                     pT4[:, 0, :],
                                                     dmask[:, o, :])
                            a0 = grp[0] - _qts[0]
                            asl = _acc[:, a0:a0 + nq, :]
                            if jc == 0:
                                nc.vector.tensor_copy(asl, pT4[:, 0:nq, :])
                            else:
                                nc.vector.tensor_add(asl, asl, pT4[:, 0:nq, :])
                            for i, qt in enumerate(grp):
                                nc.tensor.matmul(
                                    pvs[qt - _qts[0]][:],
                                    vS[_b][:, jc, _hl * D:(_hl + 1) * D],
                                    pT4[:, i, :],
                                    start=(jc == 0), stop=(jc == 4 * qt + 3))
                            # eager per-qt normalize once a qt completes
                            for i, qt in enumerate(grp):
                                if jc == 4 * qt + 3:
                                    ql = qt - _qts[0]
                                    sm = pb.tile([1, 512], F32, tag="sm",
                                                 bufs=1,
                                                 name=f"sm_{_b}_{_hl}_{qt}")
                                    nc.tensor.matmul(
                                        sm[:], ones[:], _acc[:, ql, :],
                                        start=True, stop=True)
                                    rr = np_.tile([1, 512], F32, tag="rr",
                                                  bufs=2,
                                                  name=f"rr_{_b}_{_hl}_{qt}")
                                    nc.vector.reciprocal_approx_fast(
                                        rr[:], sm[:])
                                    bc = np_.tile([P, 512], F32, tag="bc",
                                                  bufs=2,
                                                  name=f"bc_{_b}_{_hl}_{qt}")
                                    nc.gpsimd.partition_broadcast(bc[:], rr[:])
                                    yt = np_.tile([P, 512], F16, tag="yt",
                                                  bufs=2,
                                                  name=f"yt_{_b}_{_hl}_{qt}")
                                    nc.vector.tensor_mul(yt[:], pvs[ql][:],
                                                         bc[:])
                                    nc.sync.dma_start(
                                        chs[_b][_hl][2 * qt][:, :],
                                        yt[:, 0:256])
                                    nc.sync.dma_start(
                                        chs[_b][_hl][2 * qt + 1][:, :],
                                        yt[:, 256:512])

                        emit_scores(0)
                        if njc > 1:
                            emit_scores(1)
                        for jc in range(njc):
                            if jc + 2 < njc:
                                emit_scores(jc + 2)
                            drain_one()

                    nc.gpsimd.collective_compute(
                        "AllToAll", mybir.AluOpType.bypass,
                        replica_groups=[list(range(NCORES))],
                        ins=[chs[b][hl].opt()], outs=[yos[b][hl].opt()],
                    )
            return last_pt[0]

        # =================== emission ===================
        phase_a(0)
        pre_a[4] = load_xt(4)     # prefetch batch 1's first x tile
        phase_b(0)
        phase_a(1)

        # prefetch wave-1 h0 output-projection weights during phase B(1)
        with tc.tile_pool(name="pwp", bufs=1) as pwp0, \
             tc.tile_pool(name="pwp2", bufs=4) as pwp2, \
             tc.tile_pool(name="ocp", bufs=3) as ocp:
            pw_h0 = pwp0.tile([P, 8, 8, P], F16, name="pw_h0")
            for co in range(8):
                nc.sync.dma_start(pw_h0[:, co, :, :],
                                  pwB_d.ap()[co][:, 0:8, :])

            pt_last = phase_b(1)

            # ---------------- Phase C: output projection -----------------
            pw_h1 = pwp0.tile([P, 8, 8, P], F16, name="pw_h1")
            for co in range(8):
                nc.sync.dma_start(pw_h1[:, co, :, :],
                                  pwB_d.ap()[co][:, 8:16, :])


            # DMA trigger queues execute strictly in order, so a
            # collective-gated gather scheduled early would head-of-line
            # block whatever sits behind it whenever an AllToAll runs long
            # (inter-core skew). Two defenses: (1) pin every gather behind
            # the LAST pair's final yt tile with dummy reads of yAB corners
            # (WAR), so the scheduler places them after all of phase B; and
            # (2) issue them from the Activation queue, whose tail (the
            # phase-C PSUM->SBUF copies) is not latency-critical.
            dummy = csp.tile([1, 4], F16, tag="dmy", name="dummy")
            for b in range(B):
                for hl in range(HPC):
                    nc.vector.tensor_add(
                        dummy[:, 2 * b + hl:2 * b + hl + 1],
                        yAB[0:1, 8 * hl, b * 256:b * 256 + 1],
                        pt_last[0:1, 0, 0:1])

            def gather(b, hl):
                yv = yos[b][hl].rearrange("(s p) t -> p s t", p=P)
                nc.scalar.dma_start(
                    yAB[:, 8 * hl:8 * (hl + 1), b * 256:(b + 1) * 256],
                    yv[:, :, :])

            gather(0, 0)
            gather(1, 0)
            with tc.tile_pool(name="pc_ps", bufs=1, space="PSUM") as pc:
                psos = []
                for co in range(8):
                    pso = pc.tile([P, 512], F32, tag="fo", bufs=8,
                                  name=f"pso_{co}")
                    psos.append(pso)
                    for kc in range(8):      # heads 0-7 (hl=0)
                        nc.tensor.matmul(pso[:], pw_h0[:, co, kc, :],
                                         yAB[:, kc, :],
                                         start=(kc == 0), stop=False)
                gather(0, 1)
                gather(1, 1)
                for co in range(8):
                    pso = psos[co]
                    for kc in range(8, 16):  # heads 8-15 (hl=1)
                        nc.tensor.matmul(pso[:], pw_h1[:, co, kc - 8, :],
                                         yAB[:, kc, :],
                                         start=False, stop=(kc == KC - 1))
                    oo = ocp.tile([P, 512], F16, tag="oo", name=f"oo_{co}")
                    nc.scalar.copy(oo[:], pso[:])
                    nc.sync.dma_start(outT_d.ap()[co * P:(co + 1) * P, :],
                                      oo[:])
                for co in range(8, KC):
                    pw = pwp2.tile([P, KC, P], F16, tag="pw", name=f"pw_{co}")
                    nc.sync.dma_start(pw[:], pwB_d.ap()[co])
                    pso = pc.tile([P, 512], F32, tag="fo", bufs=8,
                                  name=f"pso_{co}")
                    for kc in range(KC):
                        nc.tensor.matmul(pso[:], pw[:, kc, :], yAB[:, kc, :],
                                         start=(kc == 0), stop=(kc == KC - 1))
                    oo = ocp.tile([P, 512], F16, tag="oo", name=f"oo_{co}")
                    nc.scalar.copy(oo[:], pso[:])
                    nc.sync.dma_start(outT_d.ap()[co * P:(co + 1) * P, :],
                                      oo[:])

    nc.compile()
    return nc


def _host_reference(x, weights, cos, sin, mask, use_lora):
    """Numpy fallback for inputs outside the optimized assumptions."""
    (q_w, q_b, q_A, q_B, k_w, k_b, k_A, k_B,
     v_w, v_b, v_A, v_B, p_w, p_b, p_A, p_B) = weights

    def lin(xx, w, b, A, Bm):
        out = xx @ w.T + b
        if use_lora:
            out = out + (xx @ A) @ Bm
        return out

    def rope(t):
        x1, x2 = t[..., ::2], t[..., 1::2]
        y = np.stack((x1 * cos - x2 * sin, x1 * sin + x2 * cos), axis=-1)
        return y.reshape(t.shape)

    Bs, Tl, Cd = x.shape
    q = lin(x, q_w, q_b, q_A, q_B).reshape(Bs, Tl, H, D).transpose(0, 2, 1, 3)
    k = lin(x, k_w, k_b, k_A, k_B).reshape(Bs, Tl, H, D).transpose(0, 2, 1, 3)
    v = lin(x, v_w, v_b, v_A, v_B).reshape(Bs, Tl, H, D).transpose(0, 2, 1, 3)
    q, k = rope(q), rope(k)
    s = np.einsum('bhqd,bhkd->bhqk', q, k) / np.sqrt(D)
    s = np.where(mask, s, -np.inf)
    s = s - s.max(axis=-1, keepdims=True)
    p = np.exp(s)
    p /= p.sum(axis=-1, keepdims=True)
    o = np.einsum('bhqk,bhkd->bhqd', p, v).transpose(0, 2, 1, 3).reshape(Bs, Tl, Cd)
    return lin(o, p_w, p_b, p_A, p_B).astype(np.float32)


def kernel(**inputs):
    x = np.asarray(inputs["x"], np.float32)
    cos = np.asarray(inputs["cos"], np.float32)
    sin = np.asarray(inputs["sin"], np.float32)
    mask = np.asarray(inputs["mask"])
    use_lora = int(np.asarray(inputs["use_lora"]))
    ws = {}
    for nm in ("q", "k", "v", "p"):
        for suf in ("w", "b", "A", "B"):
            ws[f"{nm}_{suf}"] = np.asarray(inputs[f"{nm}_{suf}"], np.float32)

    causal = bool((mask == np.tril(np.ones((T, T), bool))).all())
    zero_bias = all(not ws[f"{nm}_b"].any() for nm in ("q", "k", "v", "p"))
    if not (causal and zero_bias and x.shape == (B, T, C)):
        weights = tuple(ws[f"{nm}_{suf}"] for nm in ("q", "k", "v", "p")
                        for suf in ("w", "b", "A", "B"))
        return _host_reference(x, weights, cos, sin, mask, use_lora)

    effT = {}
    for nm in ("q", "k", "v", "p"):
        wt = ws[f"{nm}_w"].T.copy()
        if use_lora:
            wt += ws[f"{nm}_A"] @ ws[f"{nm}_B"]
        effT[nm] = np.ascontiguousarray(wt, np.float32)

    xT = np.ascontiguousarray(x.reshape(BT, C).T)

    perm = np.concatenate([np.arange(0, D, 2), np.arange(1, D, 2)])
    cosT = cos.T.astype(np.float32)
    sinT = sin.T.astype(np.float32)
    cosA = np.tile(np.vstack([cosT, cosT]), (1, B))
    sinA = np.tile(np.vstack([-sinT, sinT]), (1, B))

    dmask = np.zeros((4, P, 512), np.float16)
    for o in range(4):
        for r in range(P):
            dmask[o, r, o * 128 + r:] = 1.0

    # output projection weight, blocked [co, p, kc, m]; kc = head index
    pwB = np.ascontiguousarray(
        effT["p"].reshape(KC, P, KC, P).transpose(2, 1, 0, 3))

    ident = np.eye(P, dtype=np.float16)

    global _PROGRAM
    if _PROGRAM is None:
        _PROGRAM = _build_program()
    nc = _PROGRAM

    in_maps = []
    for c in range(NCORES):
        # hl-major: core c owns heads c (hl=0) and c+8 (hl=1)
        cols = np.concatenate([np.arange(c * D, (c + 1) * D),
                               np.arange((c + 8) * D, (c + 9) * D)])
        wqT = effT["q"][:, cols].copy()
        wkT = effT["k"][:, cols].copy()
        for hl in range(HPC):
            sl = slice(hl * D, (hl + 1) * D)
            wqT[:, sl] = wqT[:, sl][:, perm]
            wkT[:, sl] = wkT[:, sl][:, perm]
        in_maps.append({
            "xT": xT.astype(np.float16),
            "wqT": np.ascontiguousarray(wqT).astype(np.float16),
            "wkT": np.ascontiguousarray(wkT).astype(np.float16),
            "wvT": np.ascontiguousarray(effT["v"][:, cols]).astype(np.float16),
            "pwB": pwB.astype(np.float16),
            "cosA": cosA,
            "sinA": sinA,
            "dmask": dmask,
            "ident": ident,
        })

    res = run_bass_kernel_spmd(nc, in_maps, list(range(NCORES)))

    out = np.empty((BT, C), np.float32)
    for c in range(NCORES):
        oT = res.results[c]["outT"].astype(np.float32)
        out[c * 256:(c + 1) * 256, :] = oT[:, 0:256].T
        out[T + c * 256:T + (c + 1) * 256, :] = oT[:, 256:512].T
    return out.reshape(B, T, C)

